# revision 1
# baseline (speedup 1.0000x reference)
"""Distributed permutohedral-lattice splat (scatter-add) for 8 Trainium2 cores.

Strategy (data-parallel over points, per the sharding hint):
  - Each of the 8 NeuronCores gets 1/8 of the points (padded + masked).
  - On-core: the permutohedral slot/weight math runs in f32 on the vector
    engine (op-for-op mirror of the reference, incl. the uint32 hash done in
    exact-f32 limb arithmetic mod 2^20), laid out free-major
    [128 lanes x 128 points] and PE-transposed to point-major.
  - The scatter-add runs as 4 independent serial gather-combine-scatter
    chains (chain k = simplex vertex k) into 4 per-core partial tables.
    Within a 128-row chunk, duplicate slots are merged with a selection-
    matrix matmul (rows with equal slots all receive the full sum, so
    colliding DMA writes are identical); across chunks a chain is
    serialized by the table RAW/WAW dependency; across chains the tables
    are disjoint, so no ordering is needed.
  - The 4 partial tables are summed on-device; the 8 per-core tables are
    summed on the host (the all-reduce step of the hint, folded into the
    unshard step).
"""

import os
os.environ.setdefault("NEURON_SCRATCHPAD_PAGE_SIZE", "512")
import numpy as np
from contextlib import ExitStack

import concourse.bass as bass
import concourse.tile as tile
from concourse import bacc, mybir
from concourse._compat import with_exitstack

F32 = mybir.dt.float32
I32 = mybir.dt.int32
AOT = mybir.AluOpType

D = 3
DP1 = 4
CAP = 1 << 20
MAGIC = 12582912.0            # 1.5 * 2^23 : round-to-nearest-even trick for |x| < 2^22
HMUL = 2531011
C20 = HMUL % CAP
B20 = (HMUL * HMUL) % (1 << 32) % CAP
A20 = ((HMUL * HMUL) % (1 << 32)) * HMUL % (1 << 32) % CAP
MULTS = [A20, B20, C20]       # slot = (k0*A20 + k1*B20 + k2*C20) mod 2^20
SCALES = [float(np.float32(np.sqrt(2.0 / 3.0) * DP1 / np.sqrt((i + 1.0) * (i + 2.0)))) for i in range(D)]


def build(nc, NP, n_merge_free=1024, unroll=8, gather_bufs=2, loop_mode="for_i_unrolled", z_reps=1, h_reps=1, s_reps=1, m_reps=1, cce=False):
    """NP must be a multiple of 16384. Returns nothing; program built into nc."""
    assert NP % 16384 == 0
    NT = NP // 16384              # hash tiles
    NCH = NP // 128               # point-chunks (columns in slotT/wT)

    pos = nc.dram_tensor("positions", [NP * 3], F32, kind="ExternalInput").ap()
    vals = nc.dram_tensor("values", [NP, 64], F32, kind="ExternalInput").ap()
    msk = nc.dram_tensor("mask", [NP], F32, kind="ExternalInput").ap()
    ident = nc.dram_tensor("ident", [128, 128], F32, kind="ExternalInput").ap()
    ltm = nc.dram_tensor("ltm", [128, 128], F32, kind="ExternalInput").ap()
    out = nc.dram_tensor("out", [CAP, 65], F32, kind="ExternalOutput").ap()
    tabs = [out] + [
        nc.dram_tensor(f"tab{k}", [CAP, 65], F32, kind="Internal").ap()
        for k in range(1, DP1)
    ]

    with tile.TileContext(nc) as tc:
        with ExitStack() as ctx:
            resident = ctx.enter_context(tc.tile_pool(name="resident", bufs=1))
            identity = resident.tile([128, 128], F32, tag="ident", name="ident" + '_1')
            nc.sync.dma_start(identity[:], ident[:])
            ltmask = resident.tile([128, 128], F32, tag="ltm", name="ltm")
            nc.sync.dma_start(ltmask[:], ltm[:])

            slotT_f = [resident.tile([128, NCH], F32, tag=f"sf{k}", name=f"sf{k}" + '_2') for k in range(DP1)]
            slotT_i = [resident.tile([128, NCH], I32, tag=f"si{k}", name=f"si{k}" + '_3') for k in range(DP1)]
            wT = [resident.tile([128, NCH], F32, tag=f"w{k}", name=f"w{k}" + '_4') for k in range(DP1)]

            # ---- memset all tables (incl. out: no reliance on harness zero-init) ----
            zpool = ctx.enter_context(tc.tile_pool(name="zpool", bufs=1))
            ztile = zpool.tile([128, 4096], F32, name="ztile")
            nc.vector.memset(ztile[:], 0.0)
            total = CAP * 65                      # f32 elements per table
            zchunk = 128 * 4096
            nzfull = total // zchunk              # 130 full chunks
            zrem = total - nzfull * zchunk        # remainder elements
            for _zr in range(z_reps):
             for k in range(0, DP1):
                flat = tabs[k].rearrange("v d -> (v d)")
                for i in range(nzfull):
                    nc.sync.dma_start(
                        flat[i * zchunk : (i + 1) * zchunk].rearrange("(p f) -> p f", p=128),
                        ztile[:],
                    )
                if zrem:
                    assert zrem % 128 == 0
                    nc.sync.dma_start(
                        flat[nzfull * zchunk :].rearrange("(p f) -> p f", p=128),
                        ztile[:, : zrem // 128],
                    )

            # ================= Phase H =================
            hctx = ExitStack()
            hp = hctx.enter_context(tc.tile_pool(name="hash", bufs=2))
            hpsum = hctx.enter_context(tc.tile_pool(name="hpsum", bufs=4, space="PSUM"))

            def TT(tag):
                return hp.tile([128, 128], F32, tag=tag, name=tag)

            def ts(out_, in_, s0, op0, s1=None, op1=None):
                if s1 is None:
                    nc.vector.tensor_scalar(out_, in_, s0, None, op0)
                else:
                    nc.vector.tensor_scalar(out_, in_, s0, s1, op0, op1)

            def tt(out_, a, b, op):
                nc.vector.tensor_tensor(out=out_, in0=a, in1=b, op=op)

            def stt(out_, in0, s, op0, in1, op1):
                nc.vector.scalar_tensor_tensor(out=out_, in0=in0, scalar=s, in1=in1, op0=op0, op1=op1)

            def f_round(dst, src):      # dst = rne(src), |src| < 2^22
                ts(dst, src, MAGIC, AOT.add)
                ts(dst, dst[:], MAGIC, AOT.subtract)

            for _hr in range(h_reps):
             for h in range(NT):
                ptile = hp.tile([128, 384], F32, tag="pos", name="pos" + '_5')
                nc.sync.dma_start(ptile[:], pos[h * 49152 : (h + 1) * 49152].rearrange("(p f) -> p f", p=128))
                p3 = ptile[:].rearrange("p (t c) -> p t c", c=3)

                c = [TT(f"c{i}") for i in range(3)]
                for i in range(3):
                    ts(c[i][:], p3[:, :, i], SCALES[i], AOT.mult)

                e = [TT(f"e{i}") for i in range(4)]
                # s2=c2; s1=c1+c2; s0=c0+s1; e=[s0, s1-c0, c2-2c1, -3c2]
                tt(e[1][:], c[1][:], c[2][:], AOT.add)            # e1 <- s1
                tt(e[0][:], c[0][:], e[1][:], AOT.add)            # e0 <- s0
                tt(e[1][:], e[1][:], c[0][:], AOT.subtract)       # e1 = s1 - c0
                stt(e[2][:], c[1][:], -2.0, AOT.mult, c[2][:], AOT.add)   # e2 = c2 - 2c1
                ts(e[3][:], c[2][:], -3.0, AOT.mult)              # e3 = -3c2

                rem = [TT(f"rem{i}") for i in range(4)]
                dif = [TT(f"dif{i}") for i in range(4)]
                t1 = TT("t1"); t2 = TT("t2"); t3 = TT("t3"); t4 = TT("t4")
                for i in range(4):
                    ts(t1[:], e[i][:], 0.25, AOT.mult)            # v
                    f_round(t2[:], t1[:])                          # tr
                    tt(t3[:], t2[:], t1[:], AOT.is_gt)            # tr > v
                    tt(t3[:], t2[:], t3[:], AOT.subtract)         # fl = tr - (tr>v)
                    tt(t4[:], t2[:], t1[:], AOT.is_lt)            # tr < v
                    tt(t4[:], t2[:], t4[:], AOT.add)              # ce = tr + (tr<v)
                    ts(t3[:], t3[:], 4.0, AOT.mult)               # down
                    ts(t4[:], t4[:], 4.0, AOT.mult)               # up
                    tt(t2[:], t4[:], e[i][:], AOT.subtract)       # up - e
                    tt(t1[:], e[i][:], t3[:], AOT.subtract)       # e - down
                    tt(t2[:], t2[:], t1[:], AOT.is_lt)            # pick up?
                    stt(rem[i][:], t2[:], 4.0, AOT.mult, t3[:], AOT.add)  # rem = down + 4*pick
                    tt(dif[i][:], e[i][:], rem[i][:], AOT.subtract)

                # ranks
                lt = {}
                for i in range(4):
                    for j in range(i + 1, 4):
                        lt[(i, j)] = TT(f"lt{i}{j}")
                        tt(lt[(i, j)][:], dif[i][:], dif[j][:], AOT.is_lt)
                r = [TT(f"r{i}") for i in range(4)]
                tt(r[0][:], lt[(0, 1)][:], lt[(0, 2)][:], AOT.add)
                tt(r[0][:], r[0][:], lt[(0, 3)][:], AOT.add)
                tt(r[1][:], lt[(1, 2)][:], lt[(1, 3)][:], AOT.add)
                ts(t1[:], lt[(0, 1)][:], -1.0, AOT.mult, 1.0, AOT.add)
                tt(r[1][:], r[1][:], t1[:], AOT.add)
                ts(t1[:], lt[(0, 2)][:], -1.0, AOT.mult, 2.0, AOT.add)
                tt(t1[:], t1[:], lt[(1, 2)][:], AOT.subtract)
                tt(r[2][:], t1[:], lt[(2, 3)][:], AOT.add)
                tt(t1[:], lt[(0, 3)][:], lt[(1, 3)][:], AOT.add)
                tt(t1[:], t1[:], lt[(2, 3)][:], AOT.add)
                ts(r[3][:], t1[:], -1.0, AOT.mult, 3.0, AOT.add)

                # sum_rem/4 ; shifts
                tt(t1[:], rem[0][:], rem[1][:], AOT.add)
                tt(t1[:], t1[:], rem[2][:], AOT.add)
                tt(t1[:], t1[:], rem[3][:], AOT.add)
                ts(t1[:], t1[:], 0.25, AOT.mult)                  # sum_rem
                for i in range(4):
                    tt(r[i][:], r[i][:], t1[:], AOT.add)
                for i in range(4):
                    ts(t2[:], r[i][:], 0.0, AOT.is_lt)            # rank < 0
                    ts(t3[:], r[i][:], 3.0, AOT.is_gt)            # rank > 3
                    stt(rem[i][:], t2[:], 4.0, AOT.mult, rem[i][:], AOT.add)
                    stt(rem[i][:], t3[:], -4.0, AOT.mult, rem[i][:], AOT.add)
                    stt(r[i][:], t2[:], 4.0, AOT.mult, r[i][:], AOT.add)
                    stt(r[i][:], t3[:], -4.0, AOT.mult, r[i][:], AOT.add)

                delta = [TT(f"dl{i}") for i in range(4)]
                for i in range(4):
                    tt(delta[i][:], e[i][:], rem[i][:], AOT.subtract)
                    ts(delta[i][:], delta[i][:], 0.25, AOT.mult)

                # weights: sel(r) = sum_i delta_i * (rank_i == r)
                sels = []
                for rv in range(4):
                    acc = TT(f"sel{rv}")
                    for i in range(4):
                        ts(t1[:], r[i][:], float(rv), AOT.is_equal)
                        tt(t1[:], t1[:], delta[i][:], AOT.mult)
                        if i == 0:
                            nc.vector.tensor_copy(acc[:], t1[:])
                        else:
                            tt(acc[:], acc[:], t1[:], AOT.add)
                    sels.append(acc)
                mtile = hp.tile([128, 128], F32, tag="msk", name="msk" + '_6')
                nc.sync.dma_start(mtile[:], msk[h * 16384 : (h + 1) * 16384].rearrange("(p f) -> p f", p=128))
                w = [TT(f"wv{k}") for k in range(4)]
                ts(t1[:], sels[0][:], -1.0, AOT.mult, 1.0, AOT.add)
                tt(w[0][:], sels[3][:], t1[:], AOT.add)
                tt(w[1][:], sels[2][:], sels[3][:], AOT.subtract)
                tt(w[2][:], sels[1][:], sels[2][:], AOT.subtract)
                tt(w[3][:], sels[0][:], sels[1][:], AOT.subtract)
                for k in range(4):
                    tt(w[k][:], w[k][:], mtile[:], AOT.mult)

                # keys + hash (f32 exact, mod 2^20)
                ges = {}
                for i in range(3):
                    for th in (1, 2, 3):
                        g = TT(f"ge{i}{th}")
                        ts(g[:], r[i][:], float(th), AOT.is_ge)
                        ges[(i, th)] = g

                def mod_pow2(dst, src, p2, tmp):
                    # dst = src - p2*floor(src/p2); |src| < 2^22, p2 power of two
                    ts(tmp[:], src[:], 1.0 / p2, AOT.mult)
                    f_round(dst, tmp[:])
                    tt(t4[:], dst[:], tmp[:], AOT.is_gt)
                    tt(dst[:], dst[:], t4[:], AOT.subtract)        # floor
                    stt(dst[:], dst[:], -float(p2), AOT.mult, src[:], AOT.add)

                key = TT("key"); u = TT("u"); a = TT("a"); hsum = TT("hsum"); m10 = TT("m10")
                for k in range(4):
                    for i in range(3):
                        # key_ik = rem_i + k - 4*ge(rank_i, 4-k)   (k=0 -> rem_i)
                        if k == 0:
                            src = rem[i]
                        else:
                            stt(key[:], ges[(i, 4 - k)][:], -4.0, AOT.mult, rem[i][:], AOT.add)
                            ts(key[:], key[:], float(k), AOT.add)
                            src = key
                        Ah, Al = MULTS[i] // 1024, MULTS[i] % 1024
                        ts(u[:], src[:], float(Ah), AOT.mult)      # key*Ah  (exact, <2^20)
                        mod_pow2(m10, u, 1024.0, t1)               # (key*Ah) mod 1024
                        ts(a[:], src[:], float(Al), AOT.mult)      # key*Al  (exact)
                        stt(a[:], m10[:], 1024.0, AOT.mult, a[:], AOT.add)
                        if i == 0:
                            nc.vector.tensor_copy(hsum[:], a[:])
                        else:
                            tt(hsum[:], hsum[:], a[:], AOT.add)
                    slot = TT(f"slot{k}")
                    mod_pow2(slot, hsum, float(CAP), t1)

                    # transpose slot & w to point-major and store to resident
                    pt = hpsum.tile([128, 128], F32, tag="pt", space="PSUM", name="pt_a")
                    nc.tensor.transpose(out=pt[:], in_=slot[:], identity=identity[:])
                    nc.scalar.copy(slotT_f[k][:, h * 128 : (h + 1) * 128], pt[:])
                    nc.vector.tensor_copy(slotT_i[k][:, h * 128 : (h + 1) * 128], pt[:])
                    pt2 = hpsum.tile([128, 128], F32, tag="pt", space="PSUM", name="pt_b")
                    nc.tensor.transpose(out=pt2[:], in_=w[k][:], identity=identity[:])
                    nc.scalar.copy(wT[k][:, h * 128 : (h + 1) * 128], pt2[:])

            hctx.close()

            # ================= Phase S =================
            sctx = ExitStack()
            sp = sctx.enter_context(tc.tile_pool(name="sp", bufs=4))
            gp = sctx.enter_context(tc.tile_pool(name="gp", bufs=gather_bufs))
            spsum = sctx.enter_context(tc.tile_pool(name="spsum", bufs=1, space="PSUM"))

            vals_flat = vals.rearrange("n d -> (n d)")

            def chunk_body(iv):
                vt = sp.tile([128, 64], F32, tag="vt", name="vt" + '_7')
                nc.sync.dma_start(
                    vt[:],
                    vals_flat[bass.ds(iv * 8192, 8192)].rearrange("(p f) -> p f", p=128),
                )
                for k in range(4):
                    wcol = wT[k][:, bass.ds(iv, 1)]
                    rows = sp.tile([128, 65], F32, tag=f"rows{k}", name=f"rows{k}" + '_8')
                    tt(rows[:, 0:64], vt[:], wcol.to_broadcast([128, 64]), AOT.mult)
                    nc.vector.tensor_copy(rows[:, 64:65], wcol)

                    # selection matrix (copy dynamic column to fixed tile:
                    # PE ldweights cannot take register offsets)
                    scol = sp.tile([128, 1], F32, tag=f"scol{k}", name=f"scol{k}")
                    nc.vector.tensor_copy(scol[:], slotT_f[k][:, bass.ds(iv, 1)])
                    sicol = sp.tile([128, 1], I32, tag=f"sicol{k}", name=f"sicol{k}")
                    nc.vector.tensor_copy(sicol[:], slotT_i[k][:, bass.ds(iv, 1)])
                    srow = spsum.tile([128, 128], F32, tag=f"tp{k}", space="PSUM", name=f"srow{k}")
                    nc.tensor.transpose(
                        out=srow[:],
                        in_=scol[:].to_broadcast([128, 128]),
                        identity=identity[:],
                    )
                    sel = sp.tile([128, 128], F32, tag=f"sel{k}", name=f"sel{k}" + '_9')
                    tt(sel[:], scol[:].to_broadcast([128, 128]), srow[:], AOT.is_equal)

                    acc = spsum.tile([128, 65], F32, tag=f"acc{k}", space="PSUM", name=f"acc{k}")
                    nc.tensor.matmul(out=acc[:], lhsT=sel[:], rhs=rows[:], start=True, stop=True)

                    if cce:
                        # suppress duplicate rows (their sums are already carried
                        # by the first occurrence) by pushing their offsets OOB
                        msel = sp.tile([128, 128], F32, tag=f"msel{k}", name=f"msel{k}")
                        tt(msel[:], sel[:], ltmask[:], AOT.mult)
                        cnt = sp.tile([128, 1], F32, tag=f"cnt{k}", name=f"cnt{k}")
                        nc.vector.tensor_reduce(cnt[:], msel[:], mybir.AxisListType.X, AOT.add)
                        ts(cnt[:], cnt[:], 0.0, AOT.is_gt)
                        offf = sp.tile([128, 1], F32, tag=f"offf{k}", name=f"offf{k}")
                        stt(offf[:], cnt[:], float(1 << 21), AOT.mult, scol[:], AOT.add)
                        oicol = sp.tile([128, 1], I32, tag=f"oic{k}", name=f"oic{k}")
                        nc.vector.tensor_copy(oicol[:], offf[:])
                        accs = gp.tile([128, 65], F32, tag=f"accs{k}", name=f"accs{k}")
                        nc.scalar.copy(accs[:], acc[:])
                        nc.gpsimd.indirect_dma_start(
                            out=tabs[k][:],
                            out_offset=bass.IndirectOffsetOnAxis(ap=oicol[:], axis=0),
                            in_=accs[:],
                            in_offset=None,
                            bounds_check=CAP - 1,
                            oob_is_err=False,
                            compute_op=AOT.add,
                        )
                    else:
                        cur = gp.tile([128, 65], F32, tag=f"cur{k}", name=f"cur{k}" + '_10')
                        nc.gpsimd.indirect_dma_start(
                            out=cur[:],
                            out_offset=None,
                            in_=tabs[k][:],
                            in_offset=bass.IndirectOffsetOnAxis(ap=sicol[:], axis=0),
                        )
                        new = gp.tile([128, 65], F32, tag=f"new{k}", name=f"new{k}" + '_11')
                        tt(new[:], cur[:], acc[:], AOT.add)
                        nc.gpsimd.indirect_dma_start(
                            out=tabs[k][:],
                            out_offset=bass.IndirectOffsetOnAxis(ap=sicol[:], axis=0),
                            in_=new[:],
                            in_offset=None,
                        )

            for _sr in range(s_reps):
                if loop_mode == "for_i_unrolled":
                    tc.For_i_unrolled(0, NCH, 1, chunk_body, max_unroll=unroll)
                elif loop_mode == "for_i":
                    with tc.For_i(0, NCH, 1) as _iv:
                        chunk_body(_iv)
                else:
                    for _t in range(NCH):
                        chunk_body(_t)

            sctx.close()

            # ================= Phase M =================
            mp = ctx.enter_context(tc.tile_pool(name="mp", bufs=2))
            MF = n_merge_free
            flat_out = out.rearrange("v d -> (v d)")
            flats = [t.rearrange("v d -> (v d)") for t in tabs]
            per_part = CAP * 65 // 128              # 532480
            nmt = per_part // MF
            mrem = per_part - nmt * MF
            out2d = flat_out.rearrange("(p f) -> p f", p=128)
            tabs2d = [f.rearrange("(p f) -> p f", p=128) for f in flats]
            for _mr in range(m_reps):
             for i in range(nmt + (1 if mrem else 0)):
                lo = i * MF
                hi = min((i + 1) * MF, per_part)
                w_ = hi - lo
                tin = [mp.tile([128, MF], F32, tag=f"min{_k}", name=f"min{_k}_12") for _k in range(4)]
                for k in range(4):
                    nc.sync.dma_start(tin[k][:, :w_], tabs2d[k][:, lo:hi])
                tt(tin[0][:, :w_], tin[0][:, :w_], tin[1][:, :w_], AOT.add)
                tt(tin[2][:, :w_], tin[2][:, :w_], tin[3][:, :w_], AOT.add)
                tout = mp.tile([128, MF], F32, tag="mout", name="mout" + '_13')
                tt(tout[:, :w_], tin[0][:, :w_], tin[2][:, :w_], AOT.add)
                nc.sync.dma_start(out2d[:, lo:hi], tout[:, :w_])

    return dict(NP=NP)


def make_core_inputs(pos_shard, val_shard, NP):
    """Pad a core's shard to NP points and build the input map."""
    n = pos_shard.shape[0]
    assert n <= NP
    pos = np.zeros((NP, 3), np.float32)
    pos[:n] = pos_shard
    valp = np.zeros((NP, 64), np.float32)
    valp[:n] = val_shard
    m = np.zeros((NP,), np.float32)
    m[:n] = 1.0
    return {
        "positions": pos.reshape(-1),
        "values": valp,
        "mask": m,
        "ident": np.eye(128, dtype=np.float32),
        "ltm": np.tril(np.ones((128, 128), np.float32), -1),
    }


from concourse.bass_utils import run_bass_kernel_spmd

N_CORES = 8
_CACHE = {}


def _get_program(NP):
    if NP not in _CACHE:
        nc = bacc.Bacc("TRN2", target_bir_lowering=False, debug=False, num_devices=N_CORES)
        build(nc, NP)
        nc.compile()
        _CACHE[NP] = nc
    return _CACHE[NP]


def kernel(positions, values, hash_capacity):
    positions = np.ascontiguousarray(np.asarray(positions, dtype=np.float32))
    values = np.ascontiguousarray(np.asarray(values, dtype=np.float32))
    assert int(hash_capacity) == CAP, f"kernel compiled for capacity {CAP}"
    n = positions.shape[0]
    nsh = (n + N_CORES - 1) // N_CORES
    NP = ((nsh + 16383) // 16384) * 16384

    nc = _get_program(NP)

    in_maps = []
    for c in range(N_CORES):
        lo, hi = c * nsh, min((c + 1) * nsh, n)
        in_maps.append(
            make_core_inputs(positions[lo:hi], values[lo:hi], NP)
        )

    res = run_bass_kernel_spmd(nc, in_maps, core_ids=list(range(N_CORES)))

    acc = np.zeros((CAP, 65), np.float64)
    for c in range(N_CORES):
        acc += res.results[c]["out"].astype(np.float64)
    return np.ascontiguousarray(acc.astype(np.float32))



# revision 20
# speedup vs baseline: 1.3441x; 1.3441x over previous
"""Distributed permutohedral-lattice splat (scatter-add) for 8 Trainium2 cores.

Strategy (data-parallel over points, per the sharding hint):
  - Each of the 8 NeuronCores gets 1/8 of the points (padded + masked).
  - On-core: the permutohedral slot/weight math runs in f32 on the vector
    engine (op-for-op mirror of the reference, incl. the uint32 hash done in
    exact-f32 limb arithmetic mod 2^20), laid out free-major
    [128 lanes x 128 points] and PE-transposed to point-major.
  - The scatter-add runs as 4 independent serial gather-combine-scatter
    chains (chain k = simplex vertex k) into 4 per-core partial tables.
    Within a 128-row chunk, duplicate slots are merged with a selection-
    matrix matmul (rows with equal slots all receive the full sum, so
    colliding DMA writes are identical); across chunks a chain is
    serialized by the table RAW/WAW dependency; across chains the tables
    are disjoint, so no ordering is needed.
  - The 4 partial tables are summed on-device; the 8 per-core tables are
    summed on the host (the all-reduce step of the hint, folded into the
    unshard step).
"""

import os
os.environ.setdefault("NEURON_SCRATCHPAD_PAGE_SIZE", "512")
import numpy as np
from contextlib import ExitStack

import concourse.bass as bass
import concourse.tile as tile
from concourse import bacc, mybir
from concourse._compat import with_exitstack

F32 = mybir.dt.float32
I32 = mybir.dt.int32
AOT = mybir.AluOpType

D = 3
DP1 = 4
CAP = 1 << 20
MAGIC = 12582912.0            # 1.5 * 2^23 : round-to-nearest-even trick for |x| < 2^22
HMUL = 2531011
C20 = HMUL % CAP
B20 = (HMUL * HMUL) % (1 << 32) % CAP
A20 = ((HMUL * HMUL) % (1 << 32)) * HMUL % (1 << 32) % CAP
MULTS = [A20, B20, C20]       # slot = (k0*A20 + k1*B20 + k2*C20) mod 2^20
TROWS = CAP + 128             # 128 trash rows absorb within-chunk duplicate descriptors
SCALES = [float(np.float32(np.sqrt(2.0 / 3.0) * DP1 / np.sqrt((i + 1.0) * (i + 2.0)))) for i in range(D)]


def build(nc, NP, n_merge_free=1024, unroll=8, gather_bufs=4, loop_mode="for_i_unrolled", z_reps=1, h_reps=1, s_reps=1, m_reps=1, cce=False, n_tables=4, host_merge=False, trim=False, trim_f32=False, trim_copycol=False, accs_eng="act", spsum_bufs=1):
    """NP must be a multiple of 16384. Returns nothing; program built into nc."""
    assert NP % 16384 == 0
    NT = NP // 16384              # hash tiles
    NCH = NP // 128               # point-chunks (columns in slotT/wT)

    pos = nc.dram_tensor("positions", [NP * 3], F32, kind="ExternalInput").ap()
    vals = nc.dram_tensor("values", [NP, 64], F32, kind="ExternalInput").ap()
    msk = nc.dram_tensor("mask", [NP], F32, kind="ExternalInput").ap()
    ident = nc.dram_tensor("ident", [128, 128], F32, kind="ExternalInput").ap()
    ltm = nc.dram_tensor("ltm", [128, 128], F32, kind="ExternalInput").ap()
    R = TROWS if trim else CAP
    out = nc.dram_tensor("out", [R, 65], F32, kind="ExternalOutput").ap()
    tabs = [out] + [
        nc.dram_tensor(
            f"tab{k}", [R, 65], F32,
            kind="ExternalOutput" if host_merge else "Internal",
        ).ap()
        for k in range(1, n_tables)
    ]
    pidx = nc.dram_tensor("pidx", [128, 1], F32, kind="ExternalInput").ap() if trim else None

    with tile.TileContext(nc) as tc:
        with ExitStack() as ctx:
            resident = ctx.enter_context(tc.tile_pool(name="resident", bufs=1))
            identity = resident.tile([128, 128], F32, tag="ident", name="ident" + '_1')
            nc.sync.dma_start(identity[:], ident[:])
            ltmask = resident.tile([128, 128], F32, tag="ltm", name="ltm")
            nc.sync.dma_start(ltmask[:], ltm[:])
            BF = mybir.dt.bfloat16
            ltmb = resident.tile([128, 128], BF, tag="ltmb", name="ltmb")
            nc.vector.tensor_copy(ltmb[:], ltmask[:])
            if trim:
                trashc = resident.tile([128, 1], F32, tag="trash", name="trash")
                nc.sync.dma_start(trashc[:], pidx[:])
                nc.vector.tensor_scalar(trashc[:], trashc[:], float(CAP), None, AOT.add)

            slotT_f = [resident.tile([128, NCH], F32, tag=f"sf{k}", name=f"sf{k}" + '_2') for k in range(DP1)]
            slotT_i = [resident.tile([128, NCH], I32, tag=f"si{k}", name=f"si{k}" + '_3') for k in range(DP1)]
            wT = [resident.tile([128, NCH], F32, tag=f"w{k}", name=f"w{k}" + '_4') for k in range(DP1)]

            # ---- memset all tables (incl. out: no reliance on harness zero-init) ----
            zpool = ctx.enter_context(tc.tile_pool(name="zpool", bufs=1))
            ztile = zpool.tile([128, 4096], F32, name="ztile")
            nc.vector.memset(ztile[:], 0.0)
            total = R * 65                        # f32 elements per table
            zchunk = 128 * 4096
            nzfull = total // zchunk              # 130 full chunks
            zrem = total - nzfull * zchunk        # remainder elements
            for _zr in range(z_reps):
             for k in range(0, n_tables):
                flat = tabs[k].rearrange("v d -> (v d)")
                for i in range(nzfull):
                    nc.sync.dma_start(
                        flat[i * zchunk : (i + 1) * zchunk].rearrange("(p f) -> p f", p=128),
                        ztile[:],
                    )
                if zrem:
                    assert zrem % 128 == 0
                    nc.sync.dma_start(
                        flat[nzfull * zchunk :].rearrange("(p f) -> p f", p=128),
                        ztile[:, : zrem // 128],
                    )

            # ================= Phase H =================
            hctx = ExitStack()
            hp = hctx.enter_context(tc.tile_pool(name="hash", bufs=2))
            hpsum = hctx.enter_context(tc.tile_pool(name="hpsum", bufs=4, space="PSUM"))

            def TT(tag):
                return hp.tile([128, 128], F32, tag=tag, name=tag)

            def ts(out_, in_, s0, op0, s1=None, op1=None):
                if s1 is None:
                    nc.vector.tensor_scalar(out_, in_, s0, None, op0)
                else:
                    nc.vector.tensor_scalar(out_, in_, s0, s1, op0, op1)

            def tt(out_, a, b, op):
                nc.vector.tensor_tensor(out=out_, in0=a, in1=b, op=op)

            def stt(out_, in0, s, op0, in1, op1):
                nc.vector.scalar_tensor_tensor(out=out_, in0=in0, scalar=s, in1=in1, op0=op0, op1=op1)

            def f_round(dst, src):      # dst = rne(src), |src| < 2^22
                ts(dst, src, MAGIC, AOT.add)
                ts(dst, dst[:], MAGIC, AOT.subtract)

            for _hr in range(h_reps):
             for h in range(NT):
                ptile = hp.tile([128, 384], F32, tag="pos", name="pos" + '_5')
                nc.sync.dma_start(ptile[:], pos[h * 49152 : (h + 1) * 49152].rearrange("(p f) -> p f", p=128))
                p3 = ptile[:].rearrange("p (t c) -> p t c", c=3)

                c = [TT(f"c{i}") for i in range(3)]
                for i in range(3):
                    ts(c[i][:], p3[:, :, i], SCALES[i], AOT.mult)

                e = [TT(f"e{i}") for i in range(4)]
                # s2=c2; s1=c1+c2; s0=c0+s1; e=[s0, s1-c0, c2-2c1, -3c2]
                tt(e[1][:], c[1][:], c[2][:], AOT.add)            # e1 <- s1
                tt(e[0][:], c[0][:], e[1][:], AOT.add)            # e0 <- s0
                tt(e[1][:], e[1][:], c[0][:], AOT.subtract)       # e1 = s1 - c0
                stt(e[2][:], c[1][:], -2.0, AOT.mult, c[2][:], AOT.add)   # e2 = c2 - 2c1
                ts(e[3][:], c[2][:], -3.0, AOT.mult)              # e3 = -3c2

                rem = [TT(f"rem{i}") for i in range(4)]
                dif = [TT(f"dif{i}") for i in range(4)]
                t1 = TT("t1"); t2 = TT("t2"); t3 = TT("t3"); t4 = TT("t4")
                for i in range(4):
                    ts(t1[:], e[i][:], 0.25, AOT.mult)            # v
                    f_round(t2[:], t1[:])                          # tr
                    tt(t3[:], t2[:], t1[:], AOT.is_gt)            # tr > v
                    tt(t3[:], t2[:], t3[:], AOT.subtract)         # fl = tr - (tr>v)
                    tt(t4[:], t2[:], t1[:], AOT.is_lt)            # tr < v
                    tt(t4[:], t2[:], t4[:], AOT.add)              # ce = tr + (tr<v)
                    ts(t3[:], t3[:], 4.0, AOT.mult)               # down
                    ts(t4[:], t4[:], 4.0, AOT.mult)               # up
                    tt(t2[:], t4[:], e[i][:], AOT.subtract)       # up - e
                    tt(t1[:], e[i][:], t3[:], AOT.subtract)       # e - down
                    tt(t2[:], t2[:], t1[:], AOT.is_lt)            # pick up?
                    stt(rem[i][:], t2[:], 4.0, AOT.mult, t3[:], AOT.add)  # rem = down + 4*pick
                    tt(dif[i][:], e[i][:], rem[i][:], AOT.subtract)

                # ranks
                lt = {}
                for i in range(4):
                    for j in range(i + 1, 4):
                        lt[(i, j)] = TT(f"lt{i}{j}")
                        tt(lt[(i, j)][:], dif[i][:], dif[j][:], AOT.is_lt)
                r = [TT(f"r{i}") for i in range(4)]
                tt(r[0][:], lt[(0, 1)][:], lt[(0, 2)][:], AOT.add)
                tt(r[0][:], r[0][:], lt[(0, 3)][:], AOT.add)
                tt(r[1][:], lt[(1, 2)][:], lt[(1, 3)][:], AOT.add)
                ts(t1[:], lt[(0, 1)][:], -1.0, AOT.mult, 1.0, AOT.add)
                tt(r[1][:], r[1][:], t1[:], AOT.add)
                ts(t1[:], lt[(0, 2)][:], -1.0, AOT.mult, 2.0, AOT.add)
                tt(t1[:], t1[:], lt[(1, 2)][:], AOT.subtract)
                tt(r[2][:], t1[:], lt[(2, 3)][:], AOT.add)
                tt(t1[:], lt[(0, 3)][:], lt[(1, 3)][:], AOT.add)
                tt(t1[:], t1[:], lt[(2, 3)][:], AOT.add)
                ts(r[3][:], t1[:], -1.0, AOT.mult, 3.0, AOT.add)

                # sum_rem/4 ; shifts
                tt(t1[:], rem[0][:], rem[1][:], AOT.add)
                tt(t1[:], t1[:], rem[2][:], AOT.add)
                tt(t1[:], t1[:], rem[3][:], AOT.add)
                ts(t1[:], t1[:], 0.25, AOT.mult)                  # sum_rem
                for i in range(4):
                    tt(r[i][:], r[i][:], t1[:], AOT.add)
                for i in range(4):
                    ts(t2[:], r[i][:], 0.0, AOT.is_lt)            # rank < 0
                    ts(t3[:], r[i][:], 3.0, AOT.is_gt)            # rank > 3
                    stt(rem[i][:], t2[:], 4.0, AOT.mult, rem[i][:], AOT.add)
                    stt(rem[i][:], t3[:], -4.0, AOT.mult, rem[i][:], AOT.add)
                    stt(r[i][:], t2[:], 4.0, AOT.mult, r[i][:], AOT.add)
                    stt(r[i][:], t3[:], -4.0, AOT.mult, r[i][:], AOT.add)

                delta = [TT(f"dl{i}") for i in range(4)]
                for i in range(4):
                    tt(delta[i][:], e[i][:], rem[i][:], AOT.subtract)
                    ts(delta[i][:], delta[i][:], 0.25, AOT.mult)

                # weights: sel(r) = sum_i delta_i * (rank_i == r)
                sels = []
                for rv in range(4):
                    acc = TT(f"sel{rv}")
                    for i in range(4):
                        ts(t1[:], r[i][:], float(rv), AOT.is_equal)
                        tt(t1[:], t1[:], delta[i][:], AOT.mult)
                        if i == 0:
                            nc.vector.tensor_copy(acc[:], t1[:])
                        else:
                            tt(acc[:], acc[:], t1[:], AOT.add)
                    sels.append(acc)
                mtile = hp.tile([128, 128], F32, tag="msk", name="msk" + '_6')
                nc.sync.dma_start(mtile[:], msk[h * 16384 : (h + 1) * 16384].rearrange("(p f) -> p f", p=128))
                w = [TT(f"wv{k}") for k in range(4)]
                ts(t1[:], sels[0][:], -1.0, AOT.mult, 1.0, AOT.add)
                tt(w[0][:], sels[3][:], t1[:], AOT.add)
                tt(w[1][:], sels[2][:], sels[3][:], AOT.subtract)
                tt(w[2][:], sels[1][:], sels[2][:], AOT.subtract)
                tt(w[3][:], sels[0][:], sels[1][:], AOT.subtract)
                for k in range(4):
                    tt(w[k][:], w[k][:], mtile[:], AOT.mult)

                # keys + hash (f32 exact, mod 2^20)
                ges = {}
                for i in range(3):
                    for th in (1, 2, 3):
                        g = TT(f"ge{i}{th}")
                        ts(g[:], r[i][:], float(th), AOT.is_ge)
                        ges[(i, th)] = g

                def mod_pow2(dst, src, p2, tmp):
                    # dst = src - p2*floor(src/p2); |src| < 2^22, p2 power of two
                    ts(tmp[:], src[:], 1.0 / p2, AOT.mult)
                    f_round(dst, tmp[:])
                    tt(t4[:], dst[:], tmp[:], AOT.is_gt)
                    tt(dst[:], dst[:], t4[:], AOT.subtract)        # floor
                    stt(dst[:], dst[:], -float(p2), AOT.mult, src[:], AOT.add)

                key = TT("key"); u = TT("u"); a = TT("a"); hsum = TT("hsum"); m10 = TT("m10")
                for k in range(4):
                    for i in range(3):
                        # key_ik = rem_i + k - 4*ge(rank_i, 4-k)   (k=0 -> rem_i)
                        if k == 0:
                            src = rem[i]
                        else:
                            stt(key[:], ges[(i, 4 - k)][:], -4.0, AOT.mult, rem[i][:], AOT.add)
                            ts(key[:], key[:], float(k), AOT.add)
                            src = key
                        Ah, Al = MULTS[i] // 1024, MULTS[i] % 1024
                        ts(u[:], src[:], float(Ah), AOT.mult)      # key*Ah  (exact, <2^20)
                        mod_pow2(m10, u, 1024.0, t1)               # (key*Ah) mod 1024
                        ts(a[:], src[:], float(Al), AOT.mult)      # key*Al  (exact)
                        stt(a[:], m10[:], 1024.0, AOT.mult, a[:], AOT.add)
                        if i == 0:
                            nc.vector.tensor_copy(hsum[:], a[:])
                        else:
                            tt(hsum[:], hsum[:], a[:], AOT.add)
                    slot = TT(f"slot{k}")
                    mod_pow2(slot, hsum, float(CAP), t1)

                    # transpose slot & w to point-major and store to resident
                    pt = hpsum.tile([128, 128], F32, tag="pt", space="PSUM", name="pt_a")
                    nc.tensor.transpose(out=pt[:], in_=slot[:], identity=identity[:])
                    nc.scalar.copy(slotT_f[k][:, h * 128 : (h + 1) * 128], pt[:])
                    nc.vector.tensor_copy(slotT_i[k][:, h * 128 : (h + 1) * 128], pt[:])
                    pt2 = hpsum.tile([128, 128], F32, tag="pt", space="PSUM", name="pt_b")
                    nc.tensor.transpose(out=pt2[:], in_=w[k][:], identity=identity[:])
                    nc.scalar.copy(wT[k][:, h * 128 : (h + 1) * 128], pt2[:])

            hctx.close()

            # ================= Phase S =================
            sctx = ExitStack()
            sp = sctx.enter_context(tc.tile_pool(name="sp", bufs=4))
            gp = sctx.enter_context(tc.tile_pool(name="gp", bufs=gather_bufs))
            spsum = sctx.enter_context(tc.tile_pool(name="spsum", bufs=spsum_bufs, space="PSUM"))

            vals_flat = vals.rearrange("n d -> (n d)")

            def chunk_body_trim(iv):
                # static-index, cce-only, bf16 select pipeline, scalar offload.
                # duplicate rows scatter-add ZERO (their sel row is masked to 0
                # via the first-occurrence flag) so no OOB suppression needed.
                vt = sp.tile([128, 64], F32, tag="vt", name="vt" + '_7')
                nc.sync.dma_start(
                    vt[:],
                    vals_flat[iv * 8192 : (iv + 1) * 8192].rearrange("(p f) -> p f", p=128),
                )
                for k in range(4):
                    TD = F32 if trim_f32 else BF
                    LTM = ltmask if trim_f32 else ltmb
                    if trim_copycol:
                        wcol = sp.tile([128, 1], F32, tag=f"wc{k}", name=f"wc{k}")
                        nc.vector.tensor_copy(wcol[:], wT[k][:, iv : iv + 1])
                        wcol = wcol[:]
                        scol = sp.tile([128, 1], F32, tag=f"sc{k}", name=f"sc{k}")
                        nc.vector.tensor_copy(scol[:], slotT_f[k][:, iv : iv + 1])
                        scol = scol[:]
                    else:
                        wcol = wT[k][:, iv : iv + 1]
                        scol = slotT_f[k][:, iv : iv + 1]
                    rows = sp.tile([128, 65], TD, tag=f"rows{k}", name=f"rows{k}" + '_8')
                    tt(rows[:, 0:64], vt[:], wcol.to_broadcast([128, 64]), AOT.mult)
                    nc.scalar.copy(rows[:, 64:65], wcol)
                    srow = spsum.tile([128, 128], F32, tag=f"tp{k}", space="PSUM", name=f"srow{k}")
                    nc.tensor.transpose(
                        out=srow[:],
                        in_=scol.to_broadcast([128, 128]),
                        identity=identity[:],
                    )
                    sel = sp.tile([128, 128], TD, tag=f"sel{k}", name=f"sel{k}" + '_9')
                    tt(sel[:], scol.to_broadcast([128, 128]), srow[:], AOT.is_equal)
                    msel = sp.tile([128, 128], TD, tag=f"msel{k}", name=f"msel{k}")
                    tt(msel[:], sel[:], LTM[:], AOT.mult)
                    cnt = sp.tile([128, 1], F32, tag=f"cnt{k}", name=f"cnt{k}")
                    nc.vector.tensor_reduce(cnt[:], msel[:], mybir.AxisListType.X, AOT.add)
                    # duplicate rows are redirected to trash row CAP+p: all
                    # descriptors of one indirect DMA then hit distinct rows
                    # (same-row CCE descriptors race across SDMA engines).
                    dup = sp.tile([128, 1], F32, tag=f"dup{k}", name=f"dup{k}")
                    ts(dup[:], cnt[:], 0.0, AOT.is_gt)
                    dtr = sp.tile([128, 1], F32, tag=f"dtr{k}", name=f"dtr{k}")
                    stt(dtr[:], dup[:], 1.0, AOT.mult, trashc[:], AOT.mult)
                    offf = sp.tile([128, 1], F32, tag=f"off{k}", name=f"off{k}")
                    tt(offf[:], dtr[:], scol, AOT.max)
                    oicol = sp.tile([128, 1], I32, tag=f"oic{k}", name=f"oic{k}")
                    nc.vector.tensor_copy(oicol[:], offf[:])
                    acc = spsum.tile([128, 65], F32, tag=f"acc{k}", space="PSUM", name=f"acc{k}")
                    nc.tensor.matmul(out=acc[:], lhsT=sel[:], rhs=rows[:], start=True, stop=True)
                    accs = gp.tile([128, 65], F32, tag=f"accs{k}", name=f"accs{k}")
                    if accs_eng == "act":
                        nc.scalar.copy(accs[:], acc[:])
                    else:
                        nc.vector.tensor_copy(accs[:], acc[:])
                    nc.gpsimd.indirect_dma_start(
                        out=tabs[k % n_tables][:],
                        out_offset=bass.IndirectOffsetOnAxis(ap=oicol[:], axis=0),
                        in_=accs[:],
                        in_offset=None,
                        compute_op=AOT.add,
                    )

            def chunk_body(iv):
                vt = sp.tile([128, 64], F32, tag="vt", name="vt" + '_7')
                nc.sync.dma_start(
                    vt[:],
                    vals_flat[bass.ds(iv * 8192, 8192)].rearrange("(p f) -> p f", p=128),
                )
                for k in range(4):
                    wcol = wT[k][:, bass.ds(iv, 1)]
                    rows = sp.tile([128, 65], F32, tag=f"rows{k}", name=f"rows{k}" + '_8')
                    tt(rows[:, 0:64], vt[:], wcol.to_broadcast([128, 64]), AOT.mult)
                    nc.vector.tensor_copy(rows[:, 64:65], wcol)

                    # selection matrix (copy dynamic column to fixed tile:
                    # PE ldweights cannot take register offsets)
                    scol = sp.tile([128, 1], F32, tag=f"scol{k}", name=f"scol{k}")
                    nc.vector.tensor_copy(scol[:], slotT_f[k][:, bass.ds(iv, 1)])
                    sicol = sp.tile([128, 1], I32, tag=f"sicol{k}", name=f"sicol{k}")
                    nc.vector.tensor_copy(sicol[:], slotT_i[k][:, bass.ds(iv, 1)])
                    srow = spsum.tile([128, 128], F32, tag=f"tp{k}", space="PSUM", name=f"srow{k}")
                    nc.tensor.transpose(
                        out=srow[:],
                        in_=scol[:].to_broadcast([128, 128]),
                        identity=identity[:],
                    )
                    sel = sp.tile([128, 128], F32, tag=f"sel{k}", name=f"sel{k}" + '_9')
                    tt(sel[:], scol[:].to_broadcast([128, 128]), srow[:], AOT.is_equal)

                    acc = spsum.tile([128, 65], F32, tag=f"acc{k}", space="PSUM", name=f"acc{k}")
                    nc.tensor.matmul(out=acc[:], lhsT=sel[:], rhs=rows[:], start=True, stop=True)

                    if cce:
                        # suppress duplicate rows (their sums are already carried
                        # by the first occurrence) by pushing their offsets OOB
                        msel = sp.tile([128, 128], F32, tag=f"msel{k}", name=f"msel{k}")
                        tt(msel[:], sel[:], ltmask[:], AOT.mult)
                        cnt = sp.tile([128, 1], F32, tag=f"cnt{k}", name=f"cnt{k}")
                        nc.vector.tensor_reduce(cnt[:], msel[:], mybir.AxisListType.X, AOT.add)
                        ts(cnt[:], cnt[:], 0.0, AOT.is_gt)
                        offf = sp.tile([128, 1], F32, tag=f"offf{k}", name=f"offf{k}")
                        stt(offf[:], cnt[:], float(1 << 21), AOT.mult, scol[:], AOT.add)
                        oicol = sp.tile([128, 1], I32, tag=f"oic{k}", name=f"oic{k}")
                        nc.vector.tensor_copy(oicol[:], offf[:])
                        accs = gp.tile([128, 65], F32, tag=f"accs{k}", name=f"accs{k}")
                        nc.scalar.copy(accs[:], acc[:])
                        nc.gpsimd.indirect_dma_start(
                            out=tabs[k % n_tables][:],
                            out_offset=bass.IndirectOffsetOnAxis(ap=oicol[:], axis=0),
                            in_=accs[:],
                            in_offset=None,
                            bounds_check=CAP - 1,
                            oob_is_err=False,
                            compute_op=AOT.add,
                        )
                    else:
                        cur = gp.tile([128, 65], F32, tag=f"cur{k}", name=f"cur{k}" + '_10')
                        nc.gpsimd.indirect_dma_start(
                            out=cur[:],
                            out_offset=None,
                            in_=tabs[k % n_tables][:],
                            in_offset=bass.IndirectOffsetOnAxis(ap=sicol[:], axis=0),
                        )
                        new = gp.tile([128, 65], F32, tag=f"new{k}", name=f"new{k}" + '_11')
                        tt(new[:], cur[:], acc[:], AOT.add)
                        nc.gpsimd.indirect_dma_start(
                            out=tabs[k % n_tables][:],
                            out_offset=bass.IndirectOffsetOnAxis(ap=sicol[:], axis=0),
                            in_=new[:],
                            in_offset=None,
                        )

            body = chunk_body_trim if trim else chunk_body
            if trim:
                assert loop_mode == "python" and cce
            for _sr in range(s_reps):
                if loop_mode == "for_i_unrolled":
                    tc.For_i_unrolled(0, NCH, 1, body, max_unroll=unroll)
                elif loop_mode == "for_i":
                    with tc.For_i(0, NCH, 1) as _iv:
                        body(_iv)
                else:
                    for _t in range(NCH):
                        body(_t)

            sctx.close()

            # ================= Phase M =================
            mp = ctx.enter_context(tc.tile_pool(name="mp", bufs=2))
            MF = n_merge_free
            flat_out = out.rearrange("v d -> (v d)")
            flats = [t.rearrange("v d -> (v d)") for t in tabs]
            per_part = R * 65 // 128
            nmt = per_part // MF
            mrem = per_part - nmt * MF
            out2d = flat_out.rearrange("(p f) -> p f", p=128)
            tabs2d = [f.rearrange("(p f) -> p f", p=128) for f in flats]
            for _mr in range(m_reps):
             for i in range(nmt + (1 if mrem else 0)):
                if (n_tables == 1 or host_merge) and m_reps == 1:
                    break
                lo = i * MF
                hi = min((i + 1) * MF, per_part)
                w_ = hi - lo
                tin = [mp.tile([128, MF], F32, tag=f"min{_k}", name=f"min{_k}_12") for _k in range(n_tables)]
                for k in range(n_tables):
                    nc.sync.dma_start(tin[k][:, :w_], tabs2d[k][:, lo:hi])
                if n_tables == 4:
                    tt(tin[0][:, :w_], tin[0][:, :w_], tin[1][:, :w_], AOT.add)
                    tt(tin[2][:, :w_], tin[2][:, :w_], tin[3][:, :w_], AOT.add)
                    tout = mp.tile([128, MF], F32, tag="mout", name="mout" + '_13')
                    tt(tout[:, :w_], tin[0][:, :w_], tin[2][:, :w_], AOT.add)
                elif n_tables == 2:
                    tout = mp.tile([128, MF], F32, tag="mout", name="mout" + '_13')
                    tt(tout[:, :w_], tin[0][:, :w_], tin[1][:, :w_], AOT.add)
                else:
                    tout = tin[0]
                nc.sync.dma_start(out2d[:, lo:hi], tout[:, :w_])

    return dict(NP=NP)


def make_core_inputs(pos_shard, val_shard, NP):
    """Pad a core's shard to NP points and build the input map."""
    n = pos_shard.shape[0]
    assert n <= NP
    pos = np.zeros((NP, 3), np.float32)
    pos[:n] = pos_shard
    valp = np.zeros((NP, 64), np.float32)
    valp[:n] = val_shard
    m = np.zeros((NP,), np.float32)
    m[:n] = 1.0
    return {
        "positions": pos.reshape(-1),
        "values": valp,
        "mask": m,
        "ident": np.eye(128, dtype=np.float32),
        "ltm": np.tril(np.ones((128, 128), np.float32), -1),
        "pidx": np.arange(128, dtype=np.float32).reshape(128, 1),
    }


from concourse.bass_utils import run_bass_kernel_spmd

N_CORES = 8
_CACHE = {}


BUILD_KW = dict(cce=True, n_tables=2, loop_mode="python", host_merge=True, trim=True)


def _get_program(NP):
    if NP not in _CACHE:
        nc = bacc.Bacc("TRN2", target_bir_lowering=False, debug=False, num_devices=N_CORES)
        build(nc, NP, **BUILD_KW)
        nc.compile()
        _CACHE[NP] = nc
    return _CACHE[NP]


def kernel(positions, values, hash_capacity):
    positions = np.ascontiguousarray(np.asarray(positions, dtype=np.float32))
    values = np.ascontiguousarray(np.asarray(values, dtype=np.float32))
    assert int(hash_capacity) == CAP, f"kernel compiled for capacity {CAP}"
    n = positions.shape[0]
    nsh = (n + N_CORES - 1) // N_CORES
    NP = ((nsh + 16383) // 16384) * 16384

    nc = _get_program(NP)

    in_maps = []
    for c in range(N_CORES):
        lo, hi = c * nsh, min((c + 1) * nsh, n)
        in_maps.append(
            make_core_inputs(positions[lo:hi], values[lo:hi], NP)
        )

    res = run_bass_kernel_spmd(nc, in_maps, core_ids=list(range(N_CORES)))

    acc = np.zeros((CAP, 65), np.float64)
    for c in range(N_CORES):
        for name, arr in res.results[c].items():
            acc += arr[:CAP].astype(np.float64)
    return np.ascontiguousarray(acc.astype(np.float32))

